# revision 1
# baseline (speedup 1.0000x reference)
"""Navier-Stokes PINN kernel for 8x Trainium2 NeuronCores.

Math: the reference MLP uses ReLU activations, so the network is piecewise
linear in its inputs. All second and third derivatives produced by jax AD are
exactly zero; the PDE residuals collapse to
    u = dpsi/dy,  v = -dpsi/dx,  p = MLP(z)[1],  f = dp/dx,  g = dp/dy.
Everything is computable from one forward pass plus two forward-mode tangent
streams (d/dx, d/dy) through the masked linear layers:
    A_1 = z @ Win + b_in,        H_1 = relu(A_1),  M_1 = step(A_1)
    T_1k = row_k(Win),           G_1k = M_1 * T_1k          (k in {x, y})
    A_i = H_{i-1} @ W_i + b_i,   T_ik = G_{i-1,k} @ W_i
    H_i = relu(A_i), M_i = step(A_i), G_ik = M_i * T_ik
    [u v p f g] = [H_L ; G_Lx ; G_Ly] @ Wfin + bfin
with Wfin assembled on the host from Wout columns (including the -1 sign
for v), so the 5 outputs come out of one accumulated matmul.

Layout: feature-major on chip — activations are (features, points) tiles so
every layer is lhsT.T @ rhs with lhsT = weight block, rhs = activations.
Sharding: pure data parallel, 8192 points per core, weights replicated.

Matmul dtype: float32r (single-pass reduced-precision fp32) runs 4x faster
than float32 on the PE. Walrus requires every f32r matmul operand to be
produced by an instruction that rounds to f32r, so compute producers write
through f32r-bitcast APs and DMA-landed tensors get one-time round-copies.
"""

import os

import numpy as np

NCORES = 8
N_TOTAL = 65536
NPC = N_TOTAL // NCORES  # points per core
HID = 256
NL = 4  # hidden->hidden layers (L=6 total: in + 4 hidden + out)
B = 512  # points per block
NB = NPC // B
P = 128
NH = HID // P  # feature halves

# matmul precision: "r" = float32r (fast, reduced precision),
# "f" = float32 (exact, 4x slower). Separate knobs for the forward
# stream (which determines relu masks) and the tangent streams.
FWD_MODE = os.environ.get("NS_FWD_MODE", "f")
TAN_MODE = os.environ.get("NS_TAN_MODE", "r")
POOL_OFFLOAD = os.environ.get("NS_POOL", "0") == "1"
TSPLIT = os.environ.get("NS_TSPLIT", "0") == "1"
# repeat the whole workload R times inside one NEFF (timing harness only)
REPEAT = int(os.environ.get("NS_REPEAT", "1"))

_NC_CACHE = {}


def _build(fwd_mode: str, tan_mode: str):
    import concourse.tile as tile
    from concourse import bacc, mybir

    f32 = mybir.dt.float32
    f32r = mybir.dt.float32r
    Relu = mybir.ActivationFunctionType.Relu
    Ident = mybir.ActivationFunctionType.Identity
    Copy = mybir.ActivationFunctionType.Copy
    mult = mybir.AluOpType.mult

    def rnd(ap, mode):
        return ap.bitcast(f32r) if mode == "r" else ap

    nc = bacc.Bacc(
        "TRN2",
        target_bir_lowering=False,
        debug=False,
        enable_asserts=False,
        num_devices=NCORES,
    )

    zt_d = nc.dram_tensor("zt", (3, NPC), f32, kind="ExternalInput").ap()
    win_d = nc.dram_tensor("win", (3, HID), f32, kind="ExternalInput").ap()
    wint_d = nc.dram_tensor("wint", (HID, 2), f32, kind="ExternalInput").ap()
    bin_d = nc.dram_tensor("bin", (HID, 1), f32, kind="ExternalInput").ap()
    wh_d = nc.dram_tensor("wh", (NL, HID, HID), f32, kind="ExternalInput").ap()
    bh_d = nc.dram_tensor("bh", (NL, HID, 1), f32, kind="ExternalInput").ap()
    wfin_d = nc.dram_tensor("wfin", (3 * HID, 5), f32, kind="ExternalInput").ap()
    bfin_d = nc.dram_tensor("bfin", (5, 1), f32, kind="ExternalInput").ap()
    out_d = nc.dram_tensor("out", (5, NPC), f32, kind="ExternalOutput").ap()

    need_r = fwd_mode in ("r", "c") or tan_mode == "r"

    with tile.TileContext(nc) as tc:
        with (
            tc.tile_pool(name="weights", bufs=1) as wpool,
            tc.tile_pool(name="zin", bufs=6) as zpool,
            tc.tile_pool(name="acts", bufs=4) as hpool,
            tc.tile_pool(name="tans", bufs=4) as gpool,
            tc.tile_pool(name="masks", bufs=4) as mpool,
            tc.tile_pool(name="outs", bufs=6) as opool,
            tc.tile_pool(name="psA", bufs=3, space="PSUM") as psA,
            tc.tile_pool(name="psT", bufs=4 if TSPLIT else 2, space="PSUM") as psT,
            tc.tile_pool(name="psO", bufs=1, space="PSUM") as psO,
        ):
            # ---- one-time weight staging ----
            def stage(name, shape, src_ap, rounded, resid=False):
                """DMA a weight into SBUF; optionally add an f32r round-copy
                and (for the compensated forward) a rounded residual W - Whi.

                Returns (f32_tile, rounded_tile, residual_tile).
                """
                t = wpool.tile(shape, f32, tag=name, name=name)
                nc.sync.dma_start(t[:], src_ap)
                if not rounded:
                    return t, None, None
                tr = wpool.tile(shape, f32, tag=name + "r", name=name + "r")
                nc.scalar.activation(tr[:].bitcast(f32r), t[:], Copy)
                if not resid:
                    return t, tr, None
                ts_ = wpool.tile(shape, f32, tag=name + "s", name=name + "s")
                nc.vector.tensor_tensor(
                    ts_[:].bitcast(f32r), t[:], tr[:], mybir.AluOpType.subtract
                )
                return t, tr, ts_

            def pick(trip, mode):
                t, tr, _ = trip
                return tr[:].bitcast(f32r) if mode == "r" else t[:]

            win_t = stage("win", [3, HID], win_d[:, :], fwd_mode == "r")
            # compensated forward: hidden weights need rounded + residual parts
            wh_resid = fwd_mode == "c"
            wint_t = []
            bin_t = []
            for h in range(NH):
                w = wpool.tile([P, 2], f32, tag=f"wint{h}", name=f"wint{h}")
                nc.sync.dma_start(w[:], wint_d[h * P : (h + 1) * P, :])
                wint_t.append(w)
                b = wpool.tile([P, 1], f32, tag=f"bin{h}", name=f"bin{h}")
                nc.sync.dma_start(b[:], bin_d[h * P : (h + 1) * P, :])
                bin_t.append(b)
            wh_t = {}
            bh_t = {}
            for li in range(NL):
                for k in range(NH):
                    for h in range(NH):
                        wh_t[li, k, h] = stage(
                            f"wh{li}{k}{h}",
                            [P, P],
                            wh_d[li, k * P : (k + 1) * P, h * P : (h + 1) * P],
                            need_r,
                            resid=wh_resid,
                        )
                for h in range(NH):
                    b = wpool.tile([P, 1], f32, tag=f"bh{li}{h}", name=f"bh{li}{h}")
                    nc.sync.dma_start(b[:], bh_d[li, h * P : (h + 1) * P, :])
                    bh_t[li, h] = b
            wfin_t = []
            for k in range(3 * NH):
                wfin_t.append(
                    stage(
                        f"wfin{k}",
                        [P, 5],
                        wfin_d[k * P : (k + 1) * P, :],
                        need_r,
                    )
                )
            bfin_t = wpool.tile([5, 1], f32, tag="bfin", name="bfin")
            nc.sync.dma_start(bfin_t[:], bfin_d[:, :])

            # ---- per-block pipeline ----
            sub = mybir.AluOpType.subtract

            def make_h(a, bias_ap, li, h):
                """relu + mask from PSUM A; returns (h_for_mm, mask_tile).

                In compensated mode the forward consumes an (hhi, hres) f32r
                pair; the last hidden H is rounded-only (feeds the f32r
                output matmul, no masks downstream of it to protect).
                """
                h_mode = tan_mode if li == NL - 1 else fwd_mode
                ht = hpool.tile([P, B], f32, tag=f"H{h}", name=f"H{h}")
                mt = mpool.tile([P, B], f32, tag=f"M{h}", name=f"M{h}")
                if h_mode == "c":
                    nc.scalar.activation(ht[:], a[:], Relu, bias=bias_ap)
                    if h == 0:
                        nc.scalar.sign(mt[:], ht[:])
                    else:
                        nc.vector.tensor_scalar(
                            mt[:], ht[:], 0.0, None, mybir.AluOpType.is_gt
                        )
                    hhi = hpool.tile([P, B], f32, tag=f"Hh{h}", name=f"Hh{h}")
                    nc.scalar.activation(hhi[:].bitcast(f32r), ht[:], Copy)
                    hres = hpool.tile([P, B], f32, tag=f"Hs{h}", name=f"Hs{h}")
                    eng = nc.gpsimd if POOL_OFFLOAD else nc.vector
                    eng.tensor_tensor(hres[:].bitcast(f32r), ht[:], hhi[:], sub)
                    return (hhi, hres), mt
                nc.scalar.activation(rnd(ht[:], h_mode), a[:], Relu, bias=bias_ap)
                nc.scalar.sign(mt[:], ht[:])
                return ht, mt

            for ib_rep in range(NB * REPEAT):
                ib = ib_rep % NB
                zt = zpool.tile([3, B], f32, tag="zt", name="zt")
                # gpsimd queue: don't serialize behind the weight-stage DMAs
                nc.gpsimd.dma_start(zt[:], zt_d[:, ib * B : (ib + 1) * B])
                if fwd_mode == "r":
                    zr = zpool.tile([3, B], f32, tag="zr", name="zr")
                    nc.scalar.activation(zr[:].bitcast(f32r), zt[:], Copy)
                    z_mm = zr[:].bitcast(f32r)
                else:
                    z_mm = zt[:]

                # input layer: A1 = Win.T @ z (fp32 in modes f/c), relu+mask,
                # tangent init
                Hs, Gs = [], []
                for h in range(NH):
                    a = psA.tile([P, B], f32, tag="A", name="A")
                    nc.tensor.matmul(
                        a[:],
                        pick(win_t, fwd_mode)[:, h * P : (h + 1) * P],
                        z_mm,
                        start=True,
                        stop=True,
                    )
                    hmm, mt = make_h(a, bin_t[h][:, 0:1], -1, h)
                    gt = gpool.tile([P, 2 * B], f32, tag=f"G{h}", name=f"G{h}")
                    ieng = nc.gpsimd if POOL_OFFLOAD else nc.vector
                    ieng.tensor_scalar(
                        rnd(gt[:, 0:B], tan_mode), mt[:], wint_t[h][:, 0:1], None, mult
                    )
                    ieng.tensor_scalar(
                        rnd(gt[:, B : 2 * B], tan_mode),
                        mt[:],
                        wint_t[h][:, 1:2],
                        None,
                        mult,
                    )
                    Hs.append(hmm)
                    Gs.append(gt)

                # hidden layers
                for li in range(NL):
                    nHs, nGs = [], []
                    for h in range(NH):
                        a = psA.tile([P, B], f32, tag="A", name="A")
                        if fwd_mode == "c":
                            n_mm = 3 * NH
                            i_mm = 0
                            for k in range(NH):
                                whi = wh_t[li, k, h][1][:].bitcast(f32r)
                                wres = wh_t[li, k, h][2][:].bitcast(f32r)
                                hhi = Hs[k][0][:].bitcast(f32r)
                                hres = Hs[k][1][:].bitcast(f32r)
                                for lhs, rhs in (
                                    (whi, hhi),
                                    (whi, hres),
                                    (wres, hhi),
                                ):
                                    nc.tensor.matmul(
                                        a[:],
                                        lhs,
                                        rhs,
                                        start=(i_mm == 0),
                                        stop=(i_mm == n_mm - 1),
                                    )
                                    i_mm += 1
                        else:
                            for k in range(NH):
                                nc.tensor.matmul(
                                    a[:],
                                    pick(wh_t[li, k, h], fwd_mode),
                                    rnd(Hs[k][:], fwd_mode),
                                    start=(k == 0),
                                    stop=(k == NH - 1),
                                )
                        hmm, mt = make_h(a, bh_t[li, h][:, 0:1], li, h)
                        gt = gpool.tile([P, 2 * B], f32, tag=f"G{h}", name=f"G{h}")
                        if TSPLIT:
                            for d in range(2):
                                tp1 = psT.tile([P, B], f32, tag="T", name="T")
                                for k in range(NH):
                                    nc.tensor.matmul(
                                        tp1[:],
                                        pick(wh_t[li, k, h], tan_mode),
                                        rnd(Gs[k][:, d * B : (d + 1) * B], tan_mode),
                                        start=(k == 0),
                                        stop=(k == NH - 1),
                                    )
                                nc.vector.tensor_tensor(
                                    rnd(gt[:, d * B : (d + 1) * B], tan_mode),
                                    tp1[:],
                                    mt[:],
                                    mult,
                                )
                        else:
                            tps = psT.tile([P, 2 * B], f32, tag="T", name="T")
                            for d in range(2):
                                for k in range(NH):
                                    nc.tensor.matmul(
                                        tps[:, d * B : (d + 1) * B],
                                        pick(wh_t[li, k, h], tan_mode),
                                        rnd(Gs[k][:, d * B : (d + 1) * B], tan_mode),
                                        start=(k == 0),
                                        stop=(k == NH - 1),
                                    )
                            m3 = mt[:].unsqueeze(1).broadcast_to((P, 2, B))
                            nc.vector.tensor_tensor(
                                rnd(gt[:], tan_mode).rearrange("p (d b) -> p d b", d=2),
                                tps[:].rearrange("p (d b) -> p d b", d=2),
                                m3,
                                mult,
                            )
                        nHs.append(hmm)
                        nGs.append(gt)
                    Hs, Gs = nHs, nGs

                # output layer: [H ; Gx ; Gy] @ Wfin -> (5, B)
                ops = psO.tile([5, B], f32, tag="O", name="O")
                chunks = [
                    (rnd(Hs[0][:], tan_mode), tan_mode),
                    (rnd(Hs[1][:], tan_mode), tan_mode),
                ]
                for d in range(2):
                    for h in range(NH):
                        chunks.append(
                            (rnd(Gs[h][:, d * B : (d + 1) * B], tan_mode), tan_mode)
                        )
                for k in range(6):
                    rhs_ap, mode = chunks[k]
                    nc.tensor.matmul(
                        ops[:],
                        pick(wfin_t[k], mode),
                        rhs_ap,
                        start=(k == 0),
                        stop=(k == 5),
                    )
                osb = opool.tile([5, B], f32, tag="osb", name="osb")
                nc.scalar.activation(osb[:], ops[:], Ident, bias=bfin_t[:, 0:1])
                nc.gpsimd.dma_start(out_d[:, ib * B : (ib + 1) * B], osb[:])

    nc.compile()
    return nc


def _get_nc():
    key = (FWD_MODE, TAN_MODE, REPEAT, POOL_OFFLOAD, TSPLIT)
    if key not in _NC_CACHE:
        _NC_CACHE[key] = _build(FWD_MODE, TAN_MODE)
    return _NC_CACHE[key]


def kernel(x, y, t, Win, b_in, Wh, b_h, Wout, b_out, _trace=False):
    from concourse import bass_utils

    x = np.asarray(x, np.float32)
    y = np.asarray(y, np.float32)
    t = np.asarray(t, np.float32)
    Win = np.asarray(Win, np.float32)
    b_in = np.asarray(b_in, np.float32)
    Wh = np.asarray(Wh, np.float32)
    b_h = np.asarray(b_h, np.float32)
    Wout = np.asarray(Wout, np.float32)
    b_out = np.asarray(b_out, np.float32)

    z = np.ascontiguousarray(
        np.stack([x[:, 0], y[:, 0], t[:, 0]], axis=0)
    )  # (3, N)
    wint = np.ascontiguousarray(Win[0:2, :].T)  # (HID, 2)
    binc = np.ascontiguousarray(b_in.reshape(HID, 1))
    bhc = np.ascontiguousarray(b_h.reshape(NL, HID, 1))
    wfin = np.zeros((3 * HID, 5), np.float32)
    wfin[2 * HID : 3 * HID, 0] = Wout[:, 0]  # u = dpsi/dy
    wfin[HID : 2 * HID, 1] = -Wout[:, 0]  # v = -dpsi/dx
    wfin[0:HID, 2] = Wout[:, 1]  # p
    wfin[HID : 2 * HID, 3] = Wout[:, 1]  # f = dp/dx
    wfin[2 * HID : 3 * HID, 4] = Wout[:, 1]  # g = dp/dy
    bfin = np.zeros((5, 1), np.float32)
    bfin[2, 0] = b_out[1]

    nc = _get_nc()
    in_maps = []
    for c in range(NCORES):
        in_maps.append(
            {
                "zt": np.ascontiguousarray(z[:, c * NPC : (c + 1) * NPC]),
                "win": Win,
                "wint": wint,
                "bin": binc,
                "wh": Wh,
                "bh": bhc,
                "wfin": wfin,
                "bfin": bfin,
            }
        )
    res = bass_utils.run_bass_kernel_spmd(
        nc, in_maps, core_ids=list(range(NCORES)), trace=_trace
    )
    kernel._last_results = res
    full = np.concatenate(
        [res.results[c]["out"] for c in range(NCORES)], axis=1
    )  # (5, N)
    return np.ascontiguousarray(full[:, :, None].astype(np.float32))



# revision 22
# speedup vs baseline: 1.0716x; 1.0716x over previous
"""Navier-Stokes PINN kernel for 8x Trainium2 NeuronCores.

Math: the reference MLP uses ReLU activations, so the network is piecewise
linear in its inputs. All second and third derivatives produced by jax AD are
exactly zero; the PDE residuals collapse to
    u = dpsi/dy,  v = -dpsi/dx,  p = MLP(z)[1],  f = dp/dx,  g = dp/dy.
Everything is computable from one forward pass plus two forward-mode tangent
streams (d/dx, d/dy) through the masked linear layers:
    A_1 = z @ Win + b_in,        H_1 = relu(A_1),  M_1 = step(A_1)
    T_1k = row_k(Win),           G_1k = M_1 * T_1k          (k in {x, y})
    A_i = H_{i-1} @ W_i + b_i,   T_ik = G_{i-1,k} @ W_i
    H_i = relu(A_i), M_i = step(A_i), G_ik = M_i * T_ik
    [u v p f g] = [H_L ; G_Lx ; G_Ly] @ Wfin + bfin
with Wfin assembled on the host from Wout columns (including the -1 sign
for v), so the 5 outputs come out of one accumulated matmul.

Layout: feature-major on chip — activations are (features, points) tiles so
every layer is lhsT.T @ rhs with lhsT = weight block, rhs = activations.
Sharding: pure data parallel, 8192 points per core, weights replicated.

Matmul dtype: float32r (single-pass reduced-precision fp32) runs 4x faster
than float32 on the PE. Walrus requires every f32r matmul operand to be
produced by an instruction that rounds to f32r, so compute producers write
through f32r-bitcast APs and DMA-landed tensors get one-time round-copies.
"""

import os

import numpy as np

NCORES = 8
N_TOTAL = 65536
NPC = N_TOTAL // NCORES  # points per core
HID = 256
NL = 4  # hidden->hidden layers (L=6 total: in + 4 hidden + out)
B = 512  # points per block
NB = NPC // B
P = 128
NH = HID // P  # feature halves

# matmul precision: "r" = float32r (fast, reduced precision),
# "f" = float32 (exact, 4x slower), "c" = legacy compensated,
# "c2" = compensated forward with rebalanced engine assignment.
FWD_MODE = os.environ.get("NS_FWD_MODE", "c2")
TAN_MODE = os.environ.get("NS_TAN_MODE", "r")
POOL_OFFLOAD = os.environ.get("NS_POOL", "0") == "1"
TSPLIT = os.environ.get("NS_TSPLIT", "0") == "1"
# repeat the whole workload R times inside one NEFF (timing harness only)
REPEAT = int(os.environ.get("NS_REPEAT", "1"))

_NC_CACHE = {}


def _build(fwd_mode: str, tan_mode: str):
    import concourse.tile as tile
    from concourse import bacc, mybir

    f32 = mybir.dt.float32
    f32r = mybir.dt.float32r
    Relu = mybir.ActivationFunctionType.Relu
    Ident = mybir.ActivationFunctionType.Identity
    Copy = mybir.ActivationFunctionType.Copy
    mult = mybir.AluOpType.mult

    def rnd(ap, mode):
        return ap.bitcast(f32r) if mode == "r" else ap

    nc = bacc.Bacc(
        "TRN2",
        target_bir_lowering=False,
        debug=False,
        enable_asserts=False,
        num_devices=NCORES,
    )

    zt_d = nc.dram_tensor("zt", (3, NPC), f32, kind="ExternalInput").ap()
    win_d = nc.dram_tensor("win", (3, HID), f32, kind="ExternalInput").ap()
    wint_d = nc.dram_tensor("wint", (HID, 2), f32, kind="ExternalInput").ap()
    bin_d = nc.dram_tensor("bin", (HID, 1), f32, kind="ExternalInput").ap()
    wh_d = nc.dram_tensor("wh", (NL, HID, HID), f32, kind="ExternalInput").ap()
    bh_d = nc.dram_tensor("bh", (NL, HID, 1), f32, kind="ExternalInput").ap()
    wfin_d = nc.dram_tensor("wfin", (3 * HID, 5), f32, kind="ExternalInput").ap()
    bfin_d = nc.dram_tensor("bfin", (5, 1), f32, kind="ExternalInput").ap()
    out_d = nc.dram_tensor("out", (5, NPC), f32, kind="ExternalOutput").ap()

    need_r = fwd_mode in ("r", "c") or tan_mode == "r"

    with tile.TileContext(nc) as tc:
        with (
            tc.tile_pool(name="weights", bufs=1) as wpool,
            tc.tile_pool(name="zin", bufs=6) as zpool,
            tc.tile_pool(name="acts", bufs=4) as hpool,
            tc.tile_pool(name="tans", bufs=4) as gpool,
            tc.tile_pool(name="masks", bufs=4) as mpool,
            tc.tile_pool(name="outs", bufs=6) as opool,
            tc.tile_pool(name="psA", bufs=3, space="PSUM") as psA,
            tc.tile_pool(name="psT", bufs=4 if TSPLIT else 2, space="PSUM") as psT,
            tc.tile_pool(name="psO", bufs=1, space="PSUM") as psO,
        ):
            # ---- one-time weight staging ----
            def stage(name, shape, src_ap, rounded, resid=False):
                """DMA a weight into SBUF; optionally add an f32r round-copy
                and (for the compensated forward) a rounded residual W - Whi.

                Returns (f32_tile, rounded_tile, residual_tile).
                """
                t = wpool.tile(shape, f32, tag=name, name=name)
                nc.sync.dma_start(t[:], src_ap)
                if not rounded:
                    return t, None, None
                tr = wpool.tile(shape, f32, tag=name + "r", name=name + "r")
                nc.scalar.activation(tr[:].bitcast(f32r), t[:], Copy)
                if not resid:
                    return t, tr, None
                ts_ = wpool.tile(shape, f32, tag=name + "s", name=name + "s")
                nc.vector.tensor_tensor(
                    ts_[:].bitcast(f32r), t[:], tr[:], mybir.AluOpType.subtract
                )
                return t, tr, ts_

            def pick(trip, mode):
                t, tr, _ = trip
                return tr[:].bitcast(f32r) if mode == "r" else t[:]

            win_t = stage("win", [3, HID], win_d[:, :], fwd_mode == "r")
            # compensated forward: hidden weights need rounded + residual parts
            wh_resid = fwd_mode == "c"
            wint_t = []
            bin_t = []
            for h in range(NH):
                w = wpool.tile([P, 2], f32, tag=f"wint{h}", name=f"wint{h}")
                nc.sync.dma_start(w[:], wint_d[h * P : (h + 1) * P, :])
                wint_t.append(w)
                b = wpool.tile([P, 1], f32, tag=f"bin{h}", name=f"bin{h}")
                nc.sync.dma_start(b[:], bin_d[h * P : (h + 1) * P, :])
                bin_t.append(b)
            wh_t = {}
            bh_t = {}
            for li in range(NL):
                for k in range(NH):
                    for h in range(NH):
                        wh_t[li, k, h] = stage(
                            f"wh{li}{k}{h}",
                            [P, P],
                            wh_d[li, k * P : (k + 1) * P, h * P : (h + 1) * P],
                            need_r,
                            resid=wh_resid,
                        )
                for h in range(NH):
                    b = wpool.tile([P, 1], f32, tag=f"bh{li}{h}", name=f"bh{li}{h}")
                    nc.sync.dma_start(b[:], bh_d[li, h * P : (h + 1) * P, :])
                    bh_t[li, h] = b
            wfin_t = []
            for k in range(3 * NH):
                wfin_t.append(
                    stage(
                        f"wfin{k}",
                        [P, 5],
                        wfin_d[k * P : (k + 1) * P, :],
                        need_r,
                    )
                )
            bfin_t = wpool.tile([5, 1], f32, tag="bfin", name="bfin")
            nc.sync.dma_start(bfin_t[:], bfin_d[:, :])

            # ---- per-block pipeline ----
            sub = mybir.AluOpType.subtract

            def make_h(a, bias_ap, li, h):
                """relu + mask from PSUM A; returns (h_for_mm, mask_tile).

                In compensated mode the forward consumes an (hhi, hres) f32r
                pair; the last hidden H is rounded-only (feeds the f32r
                output matmul, no masks downstream of it to protect).
                """
                h_mode = tan_mode if li == NL - 1 else fwd_mode
                ht = hpool.tile([P, B], f32, tag=f"H{h}", name=f"H{h}")
                mt = mpool.tile([P, B], f32, tag=f"M{h}", name=f"M{h}")
                if h_mode == "c":
                    nc.scalar.activation(ht[:], a[:], Relu, bias=bias_ap)
                    if h == 0:
                        nc.scalar.sign(mt[:], ht[:])
                    else:
                        nc.vector.tensor_scalar(
                            mt[:], ht[:], 0.0, None, mybir.AluOpType.is_gt
                        )
                    hhi = hpool.tile([P, B], f32, tag=f"Hh{h}", name=f"Hh{h}")
                    nc.scalar.activation(hhi[:].bitcast(f32r), ht[:], Copy)
                    hres = hpool.tile([P, B], f32, tag=f"Hs{h}", name=f"Hs{h}")
                    eng = nc.gpsimd if POOL_OFFLOAD else nc.vector
                    eng.tensor_tensor(hres[:].bitcast(f32r), ht[:], hhi[:], sub)
                    return (hhi, hres), mt
                nc.scalar.activation(rnd(ht[:], h_mode), a[:], Relu, bias=bias_ap)
                nc.scalar.sign(mt[:], ht[:])
                return ht, mt

            for ib_rep in range(NB * REPEAT):
                ib = ib_rep % NB
                zt = zpool.tile([3, B], f32, tag="zt", name="zt")
                # gpsimd queue: don't serialize behind the weight-stage DMAs
                nc.gpsimd.dma_start(zt[:], zt_d[:, ib * B : (ib + 1) * B])
                if fwd_mode == "r":
                    zr = zpool.tile([3, B], f32, tag="zr", name="zr")
                    nc.scalar.activation(zr[:].bitcast(f32r), zt[:], Copy)
                    z_mm = zr[:].bitcast(f32r)
                else:
                    z_mm = zt[:]

                # input layer: A1 = Win.T @ z (fp32 in modes f/c), relu+mask,
                # tangent init
                Hs, Gs = [], []
                for h in range(NH):
                    a = psA.tile([P, B], f32, tag="A", name="A")
                    nc.tensor.matmul(
                        a[:],
                        pick(win_t, fwd_mode)[:, h * P : (h + 1) * P],
                        z_mm,
                        start=True,
                        stop=True,
                    )
                    hmm, mt = make_h(a, bin_t[h][:, 0:1], -1, h)
                    gt = gpool.tile([P, 2 * B], f32, tag=f"G{h}", name=f"G{h}")
                    ieng = nc.gpsimd if POOL_OFFLOAD else nc.vector
                    ieng.tensor_scalar(
                        rnd(gt[:, 0:B], tan_mode), mt[:], wint_t[h][:, 0:1], None, mult
                    )
                    ieng.tensor_scalar(
                        rnd(gt[:, B : 2 * B], tan_mode),
                        mt[:],
                        wint_t[h][:, 1:2],
                        None,
                        mult,
                    )
                    Hs.append(hmm)
                    Gs.append(gt)

                # hidden layers
                for li in range(NL):
                    nHs, nGs = [], []
                    for h in range(NH):
                        a = psA.tile([P, B], f32, tag="A", name="A")
                        if fwd_mode == "c":
                            n_mm = 3 * NH
                            i_mm = 0
                            for k in range(NH):
                                whi = wh_t[li, k, h][1][:].bitcast(f32r)
                                wres = wh_t[li, k, h][2][:].bitcast(f32r)
                                hhi = Hs[k][0][:].bitcast(f32r)
                                hres = Hs[k][1][:].bitcast(f32r)
                                for lhs, rhs in (
                                    (whi, hhi),
                                    (whi, hres),
                                    (wres, hhi),
                                ):
                                    nc.tensor.matmul(
                                        a[:],
                                        lhs,
                                        rhs,
                                        start=(i_mm == 0),
                                        stop=(i_mm == n_mm - 1),
                                    )
                                    i_mm += 1
                        else:
                            for k in range(NH):
                                nc.tensor.matmul(
                                    a[:],
                                    pick(wh_t[li, k, h], fwd_mode),
                                    rnd(Hs[k][:], fwd_mode),
                                    start=(k == 0),
                                    stop=(k == NH - 1),
                                )
                        hmm, mt = make_h(a, bh_t[li, h][:, 0:1], li, h)
                        gt = gpool.tile([P, 2 * B], f32, tag=f"G{h}", name=f"G{h}")
                        if TSPLIT:
                            for d in range(2):
                                tp1 = psT.tile([P, B], f32, tag="T", name="T")
                                for k in range(NH):
                                    nc.tensor.matmul(
                                        tp1[:],
                                        pick(wh_t[li, k, h], tan_mode),
                                        rnd(Gs[k][:, d * B : (d + 1) * B], tan_mode),
                                        start=(k == 0),
                                        stop=(k == NH - 1),
                                    )
                                nc.vector.tensor_tensor(
                                    rnd(gt[:, d * B : (d + 1) * B], tan_mode),
                                    tp1[:],
                                    mt[:],
                                    mult,
                                )
                        else:
                            tps = psT.tile([P, 2 * B], f32, tag="T", name="T")
                            for d in range(2):
                                for k in range(NH):
                                    nc.tensor.matmul(
                                        tps[:, d * B : (d + 1) * B],
                                        pick(wh_t[li, k, h], tan_mode),
                                        rnd(Gs[k][:, d * B : (d + 1) * B], tan_mode),
                                        start=(k == 0),
                                        stop=(k == NH - 1),
                                    )
                            m3 = mt[:].unsqueeze(1).broadcast_to((P, 2, B))
                            nc.vector.tensor_tensor(
                                rnd(gt[:], tan_mode).rearrange("p (d b) -> p d b", d=2),
                                tps[:].rearrange("p (d b) -> p d b", d=2),
                                m3,
                                mult,
                            )
                        nHs.append(hmm)
                        nGs.append(gt)
                    Hs, Gs = nHs, nGs

                # output layer: [H ; Gx ; Gy] @ Wfin -> (5, B)
                ops = psO.tile([5, B], f32, tag="O", name="O")
                chunks = [
                    (rnd(Hs[0][:], tan_mode), tan_mode),
                    (rnd(Hs[1][:], tan_mode), tan_mode),
                ]
                for d in range(2):
                    for h in range(NH):
                        chunks.append(
                            (rnd(Gs[h][:, d * B : (d + 1) * B], tan_mode), tan_mode)
                        )
                for k in range(6):
                    rhs_ap, mode = chunks[k]
                    nc.tensor.matmul(
                        ops[:],
                        pick(wfin_t[k], mode),
                        rhs_ap,
                        start=(k == 0),
                        stop=(k == 5),
                    )
                osb = opool.tile([5, B], f32, tag="osb", name="osb")
                nc.scalar.activation(osb[:], ops[:], Ident, bias=bfin_t[:, 0:1])
                nc.gpsimd.dma_start(out_d[:, ib * B : (ib + 1) * B], osb[:])

    nc.compile()
    return nc


def _build_c2():
    """Compensated-forward kernel, engine-rebalanced.

    All matmuls are f32r (1 cycle/row vs fp32's 4). Mask fidelity comes from
    compensation: every f32r-rounded tensor X is carried as a (hi, res) pair
    with hi = round_f32r(X), res = X - hi, and each forward matmul accumulates
    whi@hhi + whi@hres + wres@hhi in PSUM (the res@res term is ~u^2, dropped).
    Measured on HW this keeps rel err ~1e-3 while pure f32r masks give 2.7e-2.

    Engine split per half-layer (PE is the intended bottleneck at ~19.6us
    per 512-point block; each elementwise [128,512] op costs ~0.55-0.66us on
    scalar/DVE and ~0.8-1.1us on Pool):
      scalar (Activation): relu from PSUM (+bias), hi round-copies
      DVE: masks from PSUM via is_gt(add(A, bias), 0), tangent mask-mult
      Pool (gpsimd): residual subtracts, input-layer tangent init, z DMA
    GPSIMD cannot read PSUM (walrus verifier), so everything Pool touches
    is SBUF-resident.
    """
    import concourse.tile as tile
    from concourse import bacc, mybir

    f32 = mybir.dt.float32
    f32r = mybir.dt.float32r
    Relu = mybir.ActivationFunctionType.Relu
    Ident = mybir.ActivationFunctionType.Identity
    Copy = mybir.ActivationFunctionType.Copy
    mult = mybir.AluOpType.mult
    add = mybir.AluOpType.add
    sub = mybir.AluOpType.subtract
    is_gt = mybir.AluOpType.is_gt

    nc = bacc.Bacc(
        "TRN2",
        target_bir_lowering=False,
        debug=False,
        enable_asserts=False,
        num_devices=NCORES,
    )

    zt_d = nc.dram_tensor("zt", (3, NPC), f32, kind="ExternalInput").ap()
    win_d = nc.dram_tensor("win", (3, HID), f32, kind="ExternalInput").ap()
    wint_d = nc.dram_tensor("wint", (HID, 2), f32, kind="ExternalInput").ap()
    bin_d = nc.dram_tensor("bin", (HID, 1), f32, kind="ExternalInput").ap()
    wh_d = nc.dram_tensor("wh", (NL, HID, HID), f32, kind="ExternalInput").ap()
    bh_d = nc.dram_tensor("bh", (NL, HID, 1), f32, kind="ExternalInput").ap()
    wfin_d = nc.dram_tensor("wfin", (3 * HID, 5), f32, kind="ExternalInput").ap()
    bfin_d = nc.dram_tensor("bfin", (5, 1), f32, kind="ExternalInput").ap()
    out_d = nc.dram_tensor("out", (5, NPC), f32, kind="ExternalOutput").ap()

    with tile.TileContext(nc) as tc:
        with (
            tc.tile_pool(name="weights", bufs=1) as wpool,
            tc.tile_pool(name="zin", bufs=6) as zpool,
            tc.tile_pool(name="acts", bufs=4) as hpool,
            tc.tile_pool(name="tans", bufs=4) as gpool,
            tc.tile_pool(name="masks", bufs=4) as mpool,
            tc.tile_pool(name="outs", bufs=6) as opool,
            tc.tile_pool(name="psA", bufs=3, space="PSUM") as psA,
            tc.tile_pool(name="psT", bufs=2, space="PSUM") as psT,
            tc.tile_pool(name="psO", bufs=1, space="PSUM") as psO,
        ):
            # ---- one-time weight staging, ordered by first use ----
            def pair(name, shape, src_ap):
                """DMA a weight, derive (hi, res) f32r tiles."""
                t = wpool.tile(shape, f32, tag=name, name=name)
                nc.sync.dma_start(t[:], src_ap)
                thi = wpool.tile(shape, f32, tag=name + "h", name=name + "h")
                nc.scalar.activation(thi[:].bitcast(f32r), t[:], Copy)
                tres = wpool.tile(shape, f32, tag=name + "s", name=name + "s")
                nc.gpsimd.tensor_tensor(tres[:].bitcast(f32r), t[:], thi[:], sub)
                return t, thi, tres

            _, winhi, winres = pair("win", [3, HID], win_d[:, :])
            wint_t = []
            bin_t = []
            for h in range(NH):
                w = wpool.tile([P, 2], f32, tag=f"wint{h}", name=f"wint{h}")
                nc.sync.dma_start(w[:], wint_d[h * P : (h + 1) * P, :])
                wint_t.append(w)
                b = wpool.tile([P, 1], f32, tag=f"bin{h}", name=f"bin{h}")
                nc.sync.dma_start(b[:], bin_d[h * P : (h + 1) * P, :])
                bin_t.append(b)
            wh_t = {}
            bh_t = {}
            wh_raw = {}
            for li in range(NL):
                for k in range(NH):
                    for h in range(NH):
                        trip = pair(
                            f"wh{li}{k}{h}",
                            [P, P],
                            wh_d[li, k * P : (k + 1) * P, h * P : (h + 1) * P],
                        )
                        wh_t[li, k, h] = trip[1:]
                        if li == 0:
                            wh_raw[k, h] = trip[0]
                for h in range(NH):
                    b = wpool.tile([P, 1], f32, tag=f"bh{li}{h}", name=f"bh{li}{h}")
                    nc.sync.dma_start(b[:], bh_d[li, h * P : (h + 1) * P, :])
                    bh_t[li, h] = b
            wfin_t = []
            for k in range(3 * NH):
                t = wpool.tile([P, 5], f32, tag=f"wfin{k}", name=f"wfin{k}")
                nc.sync.dma_start(t[:], wfin_d[k * P : (k + 1) * P, :])
                tr = wpool.tile([P, 5], f32, tag=f"wfin{k}r", name=f"wfin{k}r")
                nc.scalar.activation(tr[:].bitcast(f32r), t[:], Copy)
                wfin_t.append(tr)
            bfin_t = wpool.tile([5, 1], f32, tag="bfin", name="bfin")
            nc.sync.dma_start(bfin_t[:], bfin_d[:, :])

            # L0 tangent weights pre-scaled by the input-tangent seed:
            # T(L0)[f,b] = sum_j W0[j,f]*wint_d[j]*M[j,b], so fold wint into
            # the weight rows once at staging and feed the masks directly as
            # the T(L0) rhs. This removes the per-block Ginit multiplies
            # (4 Pool ops) and one rounding from the tangent path.
            wl0 = {}
            for d in range(2):
                for k in range(NH):
                    for h in range(NH):
                        wsc = wpool.tile(
                            [P, P], f32, tag=f"wl0{d}{k}{h}", name=f"wl0{d}{k}{h}"
                        )
                        nc.gpsimd.tensor_scalar(
                            wsc[:].bitcast(f32r),
                            wh_raw[k, h][:],
                            wint_t[k][:, d : d + 1],
                            None,
                            mult,
                        )
                        wl0[d, k, h] = wsc

            # ---- per-block pipeline (2-stage software pipeline) ----
            # The input stage of block ib+1 is issued BEFORE the hidden
            # stage of block ib, so its serial z->A1->relu->hi/res chain
            # (~3.5us crossing four engines) completes while the PE chews
            # block ib's hidden layers; without this the PE stalls ~2us at
            # every block boundary waiting for the input-layer residuals.
            # Input-stage tiles use dedicated tags ("in" suffix): they stay
            # live until the NEXT block's hidden stage, and sharing a ring
            # with the hidden-layer tiles would create circular waits.
            def input_stage(ib):
                zt = zpool.tile([3, B], f32, tag="zt", name="zt", bufs=3)
                nc.scalar.dma_start(zt[:], zt_d[:, ib * B : (ib + 1) * B])
                with tc.high_priority(offset=-250):
                    zhi = zpool.tile([3, B], f32, tag="zhi", name="zhi", bufs=3)
                    nc.scalar.activation(zhi[:].bitcast(f32r), zt[:], Copy)
                    zres = zpool.tile([3, B], f32, tag="zres", name="zres", bufs=3)
                    nc.gpsimd.tensor_tensor(zres[:].bitcast(f32r), zt[:], zhi[:], sub)

                # input layer: compensated A1 = Win.T @ z, relu+mask.
                # Residual-dependent matmuls last: the res operands come off
                # the Pool queue latest, and PSUM accumulation order is free.
                Hs, Ms = [], []
                a_t = []
                for h in range(NH):
                    a = psA.tile([P, B], f32, tag="A", name="A")
                    cols = slice(h * P, (h + 1) * P)
                    for i_mm, (lhs, rhs) in enumerate(
                        (
                            (winhi, zhi),
                            (winres, zhi),
                            (winhi, zres),
                        )
                    ):
                        nc.tensor.matmul(
                            a[:],
                            lhs[:, cols].bitcast(f32r),
                            rhs[:].bitcast(f32r),
                            start=(i_mm == 0),
                            stop=(i_mm == 2),
                        )
                    a_t.append(a)
                # The elementwise package runs at a LATE priority: this stage
                # is injected mid-block (after L0 of the previous block), and
                # its ops become ready before that block's L1+ relus do. The
                # scheduler is a ready-heap keyed on priority, so without the
                # demotion these ops win the scalar/DVE engines and stall the
                # hidden-layer residual chain (~1.3us/block on the PE).
                with tc.high_priority(offset=-250):
                    for h in range(NH):
                        a = a_t[h]
                        ht = hpool.tile([P, B], f32, tag=f"Hin{h}", name=f"Hin{h}", bufs=2)
                        nc.scalar.activation(ht[:], a[:], Relu, bias=bin_t[h][:, 0:1])
                        mt = mpool.tile([P, B], f32, tag=f"Min{h}", name=f"Min{h}", bufs=2)
                        nc.vector.tensor_scalar(
                            mt[:].bitcast(f32r), a[:], bin_t[h][:, 0:1], 0.0, add, is_gt
                        )
                        hhi = hpool.tile([P, B], f32, tag=f"Hhin{h}", name=f"Hhin{h}", bufs=2)
                        nc.scalar.activation(hhi[:].bitcast(f32r), ht[:], Copy)
                        hres = hpool.tile([P, B], f32, tag=f"Hsin{h}", name=f"Hsin{h}", bufs=2)
                        nc.gpsimd.tensor_tensor(hres[:].bitcast(f32r), ht[:], hhi[:], sub)
                        Hs.append((hhi, hres))
                        Ms.append(mt)
                return Hs, Ms

            def rest_of_block(ib, Hs, Ms, inject=None):
                # hidden layers. Issue order is tuned for the in-order
                # engine queues: both halves' A matmuls go first (their
                # operands — prev layer's hi/res — are ready earliest), then
                # both halves' T matmuls (their operands — prev G — come off
                # DVE last); elementwise follows readiness order per engine.
                # `inject` (the next block's input stage) is called after
                # layer 0 so its scalar/Pool package lands in the queues
                # behind L0's relu/hi/res (which feed the critical L0->L1
                # chain) and its PE matmuls pad the T(L0)->A(L1) window;
                # the input-stage Ginit multiplies are further deferred to
                # after L2 (they are Pool-heavy and not needed until the
                # next block's T(L0)).
                inj_Hs = inj_Ms = None
                Gs = None
                for li in range(NL):
                    last = li == NL - 1
                    nHs, nGs = [], []
                    a_t, tps_t, ht_t, mt_t = [], [], [], []
                    for h in range(NH):
                        a = psA.tile([P, B], f32, tag="A", name="A")
                        # hi-operand terms first, residual terms last: the
                        # hres tiles are the latest-arriving operands (scalar
                        # relu -> scalar round -> Pool subtract), and PSUM
                        # accumulation order is free.
                        terms = []
                        for k in range(NH):
                            whi, wres = wh_t[li, k, h]
                            hhi, hres = Hs[k]
                            terms.append((whi, hhi))
                            terms.append((wres, hhi))
                        for k in range(NH):
                            whi, _ = wh_t[li, k, h]
                            _, hres = Hs[k]
                            terms.append((whi, hres))
                        for i_mm, (lhs, rhs) in enumerate(terms):
                            nc.tensor.matmul(
                                a[:],
                                lhs[:].bitcast(f32r),
                                rhs[:].bitcast(f32r),
                                start=(i_mm == 0),
                                stop=(i_mm == len(terms) - 1),
                            )
                        a_t.append(a)
                    for h in range(NH):
                        tps = psT.tile([P, 2 * B], f32, tag="T", name="T")
                        for d in range(2):
                            for k in range(NH):
                                if li == 0:
                                    lhs = wl0[d, k, h][:]
                                    rhs = Ms[k][:]
                                else:
                                    lhs = wh_t[li, k, h][0][:]
                                    rhs = Gs[k][:, d * B : (d + 1) * B]
                                nc.tensor.matmul(
                                    tps[:, d * B : (d + 1) * B],
                                    lhs.bitcast(f32r),
                                    rhs.bitcast(f32r),
                                    start=(k == 0),
                                    stop=(k == NH - 1),
                                )
                        tps_t.append(tps)
                    for h in range(NH):
                        a = a_t[h]
                        ht = hpool.tile([P, B], f32, tag=f"H{h}", name=f"H{h}", bufs=3)
                        ht_out = ht[:].bitcast(f32r) if last else ht[:]
                        nc.scalar.activation(ht_out, a[:], Relu, bias=bh_t[li, h][:, 0:1])
                        mt = mpool.tile([P, B], f32, tag=f"M{h}", name=f"M{h}", bufs=3)
                        nc.vector.tensor_scalar(
                            mt[:], a[:], bh_t[li, h][:, 0:1], 0.0, add, is_gt
                        )
                        ht_t.append(ht)
                        mt_t.append(mt)
                        if last:
                            nHs.append(ht)
                        else:
                            hhi = hpool.tile([P, B], f32, tag=f"Hh{h}", name=f"Hh{h}", bufs=3)
                            nc.scalar.activation(hhi[:].bitcast(f32r), ht[:], Copy)
                            hres = hpool.tile([P, B], f32, tag=f"Hs{h}", name=f"Hs{h}", bufs=3)
                            nc.gpsimd.tensor_tensor(
                                hres[:].bitcast(f32r), ht[:], hhi[:], sub
                            )
                            nHs.append((hhi, hres))
                    for h in range(NH):
                        gt = gpool.tile([P, 2 * B], f32, tag=f"G{h}", name=f"G{h}", bufs=3)
                        m3 = mt_t[h][:].unsqueeze(1).broadcast_to((P, 2, B))
                        nc.vector.tensor_tensor(
                            gt[:].bitcast(f32r).rearrange("p (d b) -> p d b", d=2),
                            tps_t[h][:].rearrange("p (d b) -> p d b", d=2),
                            m3,
                            mult,
                        )
                        nGs.append(gt)
                    Hs, Gs = nHs, nGs
                    if li == 0 and inject is not None:
                        inj_Hs, inj_Ms = inject()

                # output layer: [H ; Gx ; Gy] @ Wfin -> (5, B)
                ops = psO.tile([5, B], f32, tag="O", name="O")
                chunks = [Hs[0][:].bitcast(f32r), Hs[1][:].bitcast(f32r)]
                for d in range(2):
                    for h in range(NH):
                        chunks.append(
                            Gs[h][:, d * B : (d + 1) * B].bitcast(f32r)
                        )
                for k in range(6):
                    nc.tensor.matmul(
                        ops[:],
                        wfin_t[k][:].bitcast(f32r),
                        chunks[k],
                        start=(k == 0),
                        stop=(k == 5),
                    )
                osb = opool.tile([5, B], f32, tag="osb", name="osb", bufs=4)
                nc.scalar.activation(osb[:], ops[:], Ident, bias=bfin_t[:, 0:1])
                nc.scalar.dma_start(out_d[:, ib * B : (ib + 1) * B], osb[:])
                return (inj_Hs, inj_Ms) if inj_Hs is not None else None

            n_blocks = NB * REPEAT
            cur = input_stage(0)
            for idx in range(n_blocks):
                inj = None
                if idx + 1 < n_blocks:
                    nib = (idx + 1) % NB
                    inj = lambda nib=nib: input_stage(nib)  # noqa: E731
                cur = rest_of_block(idx % NB, *cur, inject=inj)

    nc.compile()
    return nc


def _get_nc():
    key = (FWD_MODE, TAN_MODE, REPEAT, POOL_OFFLOAD, TSPLIT)
    if key not in _NC_CACHE:
        if FWD_MODE == "c2":
            _NC_CACHE[key] = _build_c2()
        else:
            _NC_CACHE[key] = _build(FWD_MODE, TAN_MODE)
    return _NC_CACHE[key]


def kernel(x, y, t, Win, b_in, Wh, b_h, Wout, b_out, _trace=False):
    from concourse import bass_utils

    x = np.asarray(x, np.float32)
    y = np.asarray(y, np.float32)
    t = np.asarray(t, np.float32)
    Win = np.asarray(Win, np.float32)
    b_in = np.asarray(b_in, np.float32)
    Wh = np.asarray(Wh, np.float32)
    b_h = np.asarray(b_h, np.float32)
    Wout = np.asarray(Wout, np.float32)
    b_out = np.asarray(b_out, np.float32)

    z = np.ascontiguousarray(
        np.stack([x[:, 0], y[:, 0], t[:, 0]], axis=0)
    )  # (3, N)
    wint = np.ascontiguousarray(Win[0:2, :].T)  # (HID, 2)
    binc = np.ascontiguousarray(b_in.reshape(HID, 1))
    bhc = np.ascontiguousarray(b_h.reshape(NL, HID, 1))
    wfin = np.zeros((3 * HID, 5), np.float32)
    wfin[2 * HID : 3 * HID, 0] = Wout[:, 0]  # u = dpsi/dy
    wfin[HID : 2 * HID, 1] = -Wout[:, 0]  # v = -dpsi/dx
    wfin[0:HID, 2] = Wout[:, 1]  # p
    wfin[HID : 2 * HID, 3] = Wout[:, 1]  # f = dp/dx
    wfin[2 * HID : 3 * HID, 4] = Wout[:, 1]  # g = dp/dy
    bfin = np.zeros((5, 1), np.float32)
    bfin[2, 0] = b_out[1]

    nc = _get_nc()
    in_maps = []
    for c in range(NCORES):
        in_maps.append(
            {
                "zt": np.ascontiguousarray(z[:, c * NPC : (c + 1) * NPC]),
                "win": Win,
                "wint": wint,
                "bin": binc,
                "wh": Wh,
                "bh": bhc,
                "wfin": wfin,
                "bfin": bfin,
            }
        )
    res = bass_utils.run_bass_kernel_spmd(
        nc, in_maps, core_ids=list(range(NCORES)), trace=_trace
    )
    kernel._last_results = res
    full = np.concatenate(
        [res.results[c]["out"] for c in range(NCORES)], axis=1
    )  # (5, N)
    return np.ascontiguousarray(full[:, :, None].astype(np.float32))

